# revision 72
# baseline (speedup 1.0000x reference)
"""Trainium2 Bass kernel for batched cross-attention (nn_Attention).

Problem (hardcoded shapes):
  x_inner [8, 256, 2048], x_outer [8, 256, 2048]  (B, C, L)
  Wq/Wk/Wv [128, 256], bq/bk/bv [128]             (D, C)
  q = einsum('bcl,dc->bld', x_inner, Wq) + bq
  k = einsum('bcl,dc->bld', x_outer, Wk) + bk
  v = einsum('bcl,dc->bld', x_outer, Wv) + bv
  out = softmax(q @ k^T / sqrt(D), axis=-1) @ v   -> [8, 2048, 128]

Sharding: pure data-parallel over batch, one batch element per NeuronCore
(8 cores). No collectives.

Per-core algorithm (v2 — multi-engine balanced):
  - Inputs arrive as 4 half-L tiles [128, (c=2), 1024] bf16; the two
    h0 tiles are issued first on the sync queue so K/Q projections
    start as early as possible; h1 tiles go on gpsimd after the
    identity build.  Short PE warm-up (6 matmuls) covers the DMA wait.
  - Projections: W stationary, [128, 1024] PSUM per pair; bias fused
    into the PSUM->SBUF copy (ACT Identity-with-bias for the head
    projections, VectorE tensor_scalar_add for mid-kernel ones).
  - Attention in 2 passes over Lq chunk pairs (F=512, W2=1024).  Per
    Lk tile t: two score matmuls -> [128, 1024] PSUM; softmax exp is
    SPLIT ACROSS ENGINES: most tiles use ScalarE (Exp activation,
    scale=1/sqrt(D)); a subset go on VectorE via a Schraudolph-style
    bf16 bit-trick: int16(round(s*ALPHA + BETA)) bitcast to bf16
    approximates exp(s/sqrt(D)) within +-3.3% (validated on HW; the
    softmax quotient cancels the mean bias).  Two AV matmuls per tile
    accumulate out^T [D, 1024].
  - Denominator: pair sums of P^T tiles on VectorE (bf16), quad sums
    on GpSimd (otherwise idle), then all-ones-stationary matmuls
    broadcast the column sums into a [128, 512]-per-half PSUM
    accumulator.  Normalize with reciprocal_approx_fast + multiply,
    DMA bf16 out^T [D, L] to DRAM.
  - Tail: the final Lk tile's exp is split into halves on ScalarE and
    p14/pair6 denominator matmuls are emitted at t=15 so the last
    d += ones*p15 -> recip -> mul -> DMA chain is as short as possible.
  - The host casts x/W to bf16 on the way in and transposes/upcasts
    out^T -> [L, D] f32 on the way out (pure layout/precision prep,
    like the batch scatter/gather).
Softmax max-subtraction is skipped: scores/sqrt(D) are ~N(0,1), so
exp() cannot overflow in fp32.
"""

import numpy as np

B, C, L, D = 8, 256, 2048, 128
F = 512          # Lq chunk
HALF = L // 2    # 1024
NP = 2           # passes (pairs of Lq chunks)
W2 = 2 * F       # 1024: width of paired tiles
LT = L // 128    # 16 Lk tiles
CK = C // 128    # 2 contraction chunks
SCALE = 1.0 / float(np.sqrt(D))

# Schraudolph bf16 exp: bits = round(s*ALPHA + BETA), bitcast int16->bf16
# approximates exp(s*SCALE).  -0.04305 centers the sawtooth error (+-3.3%).
ALPHA = 128.0 * SCALE * float(np.log2(np.e))
BETA = 128.0 * (127.0 - 0.04305)

# Which Lk tiles' exp runs on VectorE (per pass).  EVEN tiles only: the
# emission order per tile is scores -> exp -> av -> pair-add, so at even
# t the VectorE exp leads that tile's tree work in the queue and releases
# the score-PSUM slot promptly (at odd t it would queue behind the
# previous pair/quad adds, stalling the PE ~1us).  Tail tiles stay on
# ScalarE (t=0/1 gate the pipeline head; t=14/15 gate the tail).
# pass-1 starts at t=4 (not t=2): its first VectorE exp would otherwise
# queue behind pass-0's normalize chain (recip/mul) still draining on
# VectorE at the pass boundary
# Offload disabled: ScalarE's 32 exps (~33us) now fit under the PE wall
# (~42us) since the per-exp cost dropped; all-ACT removes both the
# VectorE-exp slot-release stalls and the approximation error entirely.
DVE_EXP = {0: (), 1: ()}

_COMPILED = None


def _build():
    import concourse.bass as bass
    import concourse.mybir as mybir
    import concourse.tile as tile
    from concourse import bacc
    from concourse.masks import make_identity
    from contextlib import ExitStack

    F32 = mybir.dt.float32
    BF16 = mybir.dt.bfloat16
    I16 = mybir.dt.int16
    AFT = mybir.ActivationFunctionType
    ALU = mybir.AluOpType
    ts = bass.ts

    nc = bacc.Bacc("TRN2", target_bir_lowering=False, debug=False, num_devices=8)

    xi_ext = nc.declare_dram_parameter("x_inner", [C, L], BF16, isOutput=False)
    xo_ext = nc.declare_dram_parameter("x_outer", [C, L], BF16, isOutput=False)
    w_ext = nc.declare_dram_parameter("W_all", [128, 3 * CK * D], BF16, isOutput=False)
    b_ext = nc.declare_dram_parameter("b_all", [D, 3], F32, isOutput=False)
    out_ext = nc.declare_dram_parameter("out", [D, L], BF16, isOutput=True)

    with tile.TileContext(nc) as tc:
        with ExitStack() as ctx:
            const = ctx.enter_context(tc.tile_pool(name="const", bufs=1))
            xin = ctx.enter_context(tc.tile_pool(name="xin", bufs=1))
            qkv = ctx.enter_context(tc.tile_pool(name="qkv", bufs=1))
            pts = ctx.enter_context(tc.tile_pool(name="pts", bufs=16))
            work = ctx.enter_context(tc.tile_pool(name="work", bufs=3))
            ps_s = ctx.enter_context(tc.tile_pool(name="ps_s", bufs=2, space="PSUM"))
            ps_av = ctx.enter_context(tc.tile_pool(name="ps_av", bufs=2, space="PSUM"))
            ps_d = ctx.enter_context(tc.tile_pool(name="ps_d", bufs=2, space="PSUM"))

            # ---- input DMAs first: h0 tiles on the sync queue (nothing
            # else competes), h1 tiles on gpsimd after the identity build.
            # Each tile is [128, (c=2), 1024]: partition p holds rows p and
            # 128+p of the [256, L] input -> 2KB contiguous DRAM runs.
            xo_t = [xin.tile([128, CK, HALF], BF16, tag=f"xo{h}", name=f"xo{h}")
                    for h in range(2)]
            xi_t = [xin.tile([128, CK, HALF], BF16, tag=f"xi{h}", name=f"xi{h}")
                    for h in range(2)]
            # weights/bias first on scalar queue (tiny; host pre-arranged
            # W as [128, 3*2*128] so each partition is one 1.5KB run)
            w_all = const.tile([128, 3, CK, D], BF16, tag="w")
            nc.scalar.dma_start(
                out=w_all[:],
                in_=w_ext[:].rearrange("p (w j d) -> p w j d", w=3, j=CK),
            )
            b_all = const.tile([D, 3], F32, tag="b")
            nc.scalar.dma_start(out=b_all[:], in_=b_ext[:])

            # All x DMAs on the sync queue IN PRIORITY ORDER: per-HW-queue
            # FIFO serializes them, so xo-h0 gets the full HBM bandwidth
            # and lands first (concurrent issue would round-robin all four
            # and finish them together ~4x later).  The h0 tiles are split
            # by c-chunk so the first projection matmul can start as soon
            # as chunk 0 lands.
            for c in range(CK):
                nc.sync.dma_start(
                    out=xo_t[0][:, c, :],
                    in_=xo_ext[c * 128:(c + 1) * 128, 0:HALF],
                )
            for c in range(CK):
                nc.sync.dma_start(
                    out=xi_t[0][:, c, :],
                    in_=xi_ext[c * 128:(c + 1) * 128, 0:HALF],
                )
            nc.sync.dma_start(
                out=xo_t[1][:],
                in_=xo_ext[:, HALF:].rearrange("(j p) l -> p j l", p=128),
            )
            nc.sync.dma_start(
                out=xi_t[1][:],
                in_=xi_ext[:, HALF:].rearrange("(j p) l -> p j l", p=128),
            )

            # vector: constants needed by the PE warm-up
            ones = const.tile([128, 128], BF16, tag="ones")
            nc.vector.memset(ones[:], 1.0)
            warm_src = const.tile([128, F], BF16, tag="warm")
            nc.vector.memset(warm_src[:], 0.0)

            # gpsimd: identity for PE transposes
            ident = const.tile([128, 128], BF16, tag="ident")
            make_identity(nc, ident[:])

            # ---- PE warm-up: a few dummy matmuls while the first input
            # DMA is in flight so HAM un-throttles before real work.
            wp = ps_d.tile([128, F], F32, tag="d", name="warm_ps")
            for _ in range(5):
                nc.tensor.matmul(wp[:], ones[:], warm_src[:], start=True, stop=True)

            # ---- projections ----------------------------------------------
            def xs_ap(tiles, c, pr):
                return tiles[pr][:, c, :]

            def project_pair(w, b, xs, pr, tag, pool=None, bias_eng="dve",
                             bias_halves=False):
                pool = pool or ps_s
                sb = qkv.tile([128, W2], BF16, tag=f"{tag}{pr}", name=f"{tag}{pr}")

                def bias_copy(dst, src):
                    if bias_eng == "act":
                        nc.scalar.activation(
                            dst, src, AFT.Identity, bias=b_all[:, b:b + 1]
                        )
                    else:
                        nc.vector.tensor_scalar_add(dst, src, b_all[:, b:b + 1])

                if pool is ps_s:
                    ps = pool.tile([128, W2], F32, tag="s", name="proj_ps")
                    for c in range(CK):
                        for h in range(2):
                            nc.tensor.matmul(
                                ps[:, ts(h, F)],
                                w_all[:, w, c, :],
                                xs_ap(xs, c, pr)[:, ts(h, F)],
                                start=(c == 0), stop=(c == CK - 1),
                            )
                    if bias_halves:
                        # per-half bias so downstream consumers of the h0
                        # half start one PSUM-half earlier (head critical
                        # chain: Q bias -> scores t0 -> exp0)
                        for h in range(2):
                            bias_copy(sb[:, ts(h, F)], ps[:, ts(h, F)])
                    else:
                        bias_copy(sb[:], ps[:])
                else:
                    # two 1-bank halves (fits the ps_d slot size)
                    for h in range(2):
                        ps = pool.tile([128, F], F32, tag="d", name="proj_ph")
                        for c in range(CK):
                            nc.tensor.matmul(
                                ps[:],
                                w_all[:, w, c, :],
                                xs_ap(xs, c, pr)[:, ts(h, F)],
                                start=(c == 0), stop=(c == CK - 1),
                            )
                        bias_copy(sb[:, ts(h, F)], ps[:])
                return sb

            ktP, vtP, qtP = [None, None], [None, None], [None, None]
            v_sb = [None] * LT

            def make_v_tiles(g, pool, ptag):
                tp_all = pool.tile([128, 8 * 128], BF16, tag=ptag, name="tp_all")
                for j in range(8):
                    t = g * 8 + j
                    nc.tensor.transpose(
                        tp_all[:, ts(j, 128)],
                        vtP[t // 8][:, (t % 8) * 128:(t % 8 + 1) * 128],
                        ident[:],
                    )
                for j in range(8):
                    t = g * 8 + j
                    vv = qkv.tile([128, 128], BF16, tag=f"v{t}", name=f"v{t}")
                    nc.vector.tensor_copy(vv[:], tp_all[:, ts(j, 128)])
                    v_sb[t] = vv

            def kslice(t):
                return ktP[t // 8][:, (t % 8) * 128:(t % 8 + 1) * 128]

            # Pair-0 projections: K first (earliest DMA), then Q (gated on
            # xi h0), then V + transposes.  K/Q bias on ScalarE (idle in
            # the head), V bias on VectorE.
            # K bias on VectorE (idle in the head) so ScalarE only carries
            # the Q bias before the first exp; the V projection/transposes
            # are emitted after the first score tile (AV first needs V at
            # t=1), shortening the critical chain to exp0.
            ktP[0] = project_pair(1, 1, xo_t, 0, "kt_", bias_eng="dve",
                                  bias_halves=True)
            qtP[0] = project_pair(0, 0, xi_t, 0, "qt_", bias_eng="act",
                                  bias_halves=True)

            # ---- attention: 2 passes over Lq chunk pairs -------------------
            state = {}

            def init_pass(pr):
                state[pr] = dict(
                    av=[ps_av.tile([128, F], F32, tag="av", name=f"av{h}")
                        for h in range(2)],
                    d=None, p_tiles=[], pair_sums=[], quad_sums=[],
                )

            def do_av(pr, t, only_h=None):
                st = state[pr]
                for h in range(2) if only_h is None else (only_h,):
                    nc.tensor.matmul(
                        st["av"][h][:], v_sb[t][:],
                        st["p_tiles"][t][:, ts(h, F)],
                        start=(t == 0), stop=(t == LT - 1),
                    )

            def do_pair_add(pr, m):
                st = state[pr]
                # pair sums on VectorE (they recycle P-tile pool slots, so
                # they must keep up with the exp rate); quad sums on GpSimd
                # (otherwise idle, and the PE d-matmuls that consume them
                # are scheduled 2+ tiles later).  The pass-0 boundary quad
                # stays on VectorE so the pass-0 normalize isn't delayed.
                sm = pts.tile([128, W2], BF16, tag="p", name="sm")
                nc.vector.tensor_add(
                    sm[:], st["p_tiles"][2 * m][:], st["p_tiles"][2 * m + 1][:]
                )
                st["pair_sums"].append(sm)
                if m % 2 == 1 and not (pr == 1 and m >= LT // 2 - 2):
                    # quads on VectorE: 0.84us right behind the pair add,
                    # so the PE d-matmuls 2 tiles later never stall (GpSimd
                    # at 2.1us/add left them ~1us late each)
                    q = pts.tile([128, W2], BF16, tag="p", name="quad")
                    nc.vector.tensor_add(
                        q[:], st["pair_sums"][m - 1][:], st["pair_sums"][m][:]
                    )
                    st["quad_sums"].append(q)

            def do_dn(pr, m):
                st = state[pr]
                if st["d"] is None:
                    st["d"] = [ps_d.tile([128, F], F32, tag="d", name=f"d{h}")
                               for h in range(2)]
                for h in range(2):
                    nc.tensor.matmul(
                        st["d"][h][:], ones[:], st["quad_sums"][m][:, ts(h, F)],
                        start=(m == 0), stop=(pr == 0 and m == LT // 4 - 1),
                    )

            def do_exp(pr, t, p_sb, s_ps):
                if t in DVE_EXP[pr]:
                    nc.vector.tensor_scalar(
                        out=p_sb[:].bitcast(I16), in0=s_ps[:],
                        scalar1=ALPHA, scalar2=BETA,
                        op0=ALU.mult, op1=ALU.add,
                    )
                elif (pr == 1 and t == LT - 1) or (pr == 0 and t == 0):
                    # split the boundary exps: at the tail the h0
                    # denominator chain starts one half-exp earlier; at the
                    # head exp0-h0 starts as soon as the h0 scores (gated
                    # on the per-half Q bias) are ready
                    for h in range(2):
                        nc.scalar.activation(
                            p_sb[:, ts(h, F)], s_ps[:, ts(h, F)],
                            AFT.Exp, scale=SCALE,
                        )
                else:
                    nc.scalar.activation(p_sb[:], s_ps[:], AFT.Exp, scale=SCALE)

            def emit_att(pr, t_lo, t_hi):
                st = state[pr]
                for t in range(t_lo, t_hi):
                    s_ps = ps_s.tile([128, W2], F32, tag="s", name="s_ps")
                    for h in range(2):
                        nc.tensor.matmul(
                            s_ps[:, ts(h, F)], kslice(t), qtP[pr][:, ts(h, F)],
                            start=True, stop=True,
                        )
                    p_sb = pts.tile([128, W2], BF16, tag="p", name="p_sb")
                    do_exp(pr, t, p_sb, s_ps)
                    st["p_tiles"].append(p_sb)
                    if t >= 1:
                        do_av(pr, t - 1)
                    if t >= 2 and t % 2 == 0:
                        do_pair_add(pr, t // 2 - 1)
                    if pr == 1:
                        # quad m is formed at t = 4m + 4 on VectorE; consume
                        # it ONE tile later so the PE d-matmul never queues
                        # behind the same-iteration pair->quad chain
                        if t >= 5 and (t - 5) % 4 == 0 and (t - 5) // 4 < 3:
                            do_dn(pr, (t - 5) // 4)
                        if t == LT - 1:
                            # pair6 (tiles 12,13) and p14 feed the
                            # denominator now so only p15 remains at the end
                            for h in range(2):
                                nc.tensor.matmul(
                                    st["d"][h][:], ones[:],
                                    st["pair_sums"][LT // 2 - 2][:, ts(h, F)],
                                    start=False, stop=False,
                                )
                            for h in range(2):
                                nc.tensor.matmul(
                                    st["d"][h][:], ones[:],
                                    st["p_tiles"][LT - 2][:, ts(h, F)],
                                    start=False, stop=False,
                                )

            def normalize_half(pr, h, st, chunks=1):
                recip = work.tile([128, F], F32, tag="recip", name="recip")
                avn = work.tile([128, F], BF16, tag="avn", name="avn")
                nc.vector.reciprocal_approx_fast(recip[:], st["d"][h][:])
                cw = F // chunks
                for i in range(chunks):
                    nc.vector.tensor_mul(
                        avn[:, ts(i, cw)],
                        st["av"][h][:, ts(i, cw)], recip[:, ts(i, cw)],
                    )
                    base = (2 * pr + h) * F + i * cw
                    nc.sync.dma_start(
                        out=out_ext[:, base:base + cw], in_=avn[:, ts(i, cw)]
                    )

            def finish_pass(pr):
                st = state[pr]
                do_av(pr, LT - 1)
                if pr == 0:
                    do_pair_add(pr, LT // 2 - 1)
                    # all of pass-0's denominator matmuls land here: they
                    # fill the pass-boundary PE gap and overlap pass-1.
                    for m in range(LT // 4):
                        do_dn(pr, m)
                    for h in range(2):
                        normalize_half(pr, h, st)
                else:
                    # tail: only p15 remains; per-half add -> d -> recip ->
                    # mul -> DMA, h0 first, with the mul/DMA split in two
                    # chunks so the output transfer overlaps the last mul
                    for h in range(2):
                        nc.tensor.matmul(
                            st["d"][h][:], ones[:],
                            st["p_tiles"][LT - 1][:, ts(h, F)],
                            start=False, stop=True,
                        )
                        normalize_half(pr, h, st, chunks=2)

            # mid-kernel projection biases ride on ScalarE (62%-busy vs
            # VectorE's denominator-tree load) as Identity-with-bias
            init_pass(0)
            emit_att(0, 0, 1)
            vtP[0] = project_pair(2, 2, xo_t, 0, "vt_", bias_eng="dve")
            make_v_tiles(0, ps_d, "d")
            emit_att(0, 1, 8)
            ktP[1] = project_pair(1, 1, xo_t, 1, "kt_", pool=ps_d, bias_eng="dve")
            vtP[1] = project_pair(2, 2, xo_t, 1, "vt_", pool=ps_d, bias_eng="dve")
            make_v_tiles(1, ps_d, "d")
            emit_att(0, 8, LT)
            qtP[1] = project_pair(0, 0, xi_t, 1, "qt_", bias_eng="dve")
            finish_pass(0)
            init_pass(1)
            emit_att(1, 0, LT)
            finish_pass(1)

    nc.compile()
    return nc


def _in_maps(inputs):
    import ml_dtypes

    bf16 = ml_dtypes.bfloat16
    x_inner = np.ascontiguousarray(np.asarray(inputs["x_inner"]).astype(bf16))
    x_outer = np.ascontiguousarray(np.asarray(inputs["x_outer"]).astype(bf16))
    # [w, c, d] -> [p, w, j, d] with c = j*128 + p, flattened to [128, 768]
    # so the on-device tile DMA reads one contiguous 1.5KB run per partition
    w_wcd = np.stack([
        np.asarray(inputs["Wq"]).astype(np.float32).T,
        np.asarray(inputs["Wk"]).astype(np.float32).T,
        np.asarray(inputs["Wv"]).astype(np.float32).T,
    ])
    w_all = np.ascontiguousarray(
        w_wcd.reshape(3, CK, 128, D).transpose(2, 0, 1, 3).reshape(128, 3 * CK * D)
    ).astype(bf16)
    b_all = np.ascontiguousarray(np.stack([
        np.asarray(inputs["bq"], dtype=np.float32),
        np.asarray(inputs["bk"], dtype=np.float32),
        np.asarray(inputs["bv"], dtype=np.float32),
    ], axis=1))
    return [
        {
            "x_inner": x_inner[b],
            "x_outer": x_outer[b],
            "W_all": w_all,
            "b_all": b_all,
        }
        for b in range(B)
    ]


def kernel(**inputs):
    global _COMPILED
    from concourse.bass_utils import run_bass_kernel_spmd

    if _COMPILED is None:
        _COMPILED = _build()
    in_maps = _in_maps(inputs)
    res = run_bass_kernel_spmd(_COMPILED, in_maps, core_ids=list(range(B)))
    # device emits bf16 out^T [D, L]; transpose/upcast on host (pure layout)
    return np.stack(
        [res.results[b]["out"].T.astype(np.float32) for b in range(B)]
    )


# revision 73
# speedup vs baseline: 1.0095x; 1.0095x over previous
"""Trainium2 Bass kernel for batched cross-attention (nn_Attention).

Problem (hardcoded shapes):
  x_inner [8, 256, 2048], x_outer [8, 256, 2048]  (B, C, L)
  Wq/Wk/Wv [128, 256], bq/bk/bv [128]             (D, C)
  q = einsum('bcl,dc->bld', x_inner, Wq) + bq
  k = einsum('bcl,dc->bld', x_outer, Wk) + bk
  v = einsum('bcl,dc->bld', x_outer, Wv) + bv
  out = softmax(q @ k^T / sqrt(D), axis=-1) @ v   -> [8, 2048, 128]

Sharding: pure data-parallel over batch, one batch element per NeuronCore
(8 cores). No collectives.

Per-core algorithm (v2 — multi-engine balanced):
  - Inputs arrive as 4 half-L tiles [128, (c=2), 1024] bf16; the two
    h0 tiles are issued first on the sync queue so K/Q projections
    start as early as possible; h1 tiles go on gpsimd after the
    identity build.  Short PE warm-up (6 matmuls) covers the DMA wait.
  - Projections: W stationary, [128, 1024] PSUM per pair; bias fused
    into the PSUM->SBUF copy (ACT Identity-with-bias for the head
    projections, VectorE tensor_scalar_add for mid-kernel ones).
  - Attention in 2 passes over Lq chunk pairs (F=512, W2=1024).  Per
    Lk tile t: two score matmuls -> [128, 1024] PSUM; softmax exp is
    SPLIT ACROSS ENGINES: most tiles use ScalarE (Exp activation,
    scale=1/sqrt(D)); a subset go on VectorE via a Schraudolph-style
    bf16 bit-trick: int16(round(s*ALPHA + BETA)) bitcast to bf16
    approximates exp(s/sqrt(D)) within +-3.3% (validated on HW; the
    softmax quotient cancels the mean bias).  Two AV matmuls per tile
    accumulate out^T [D, 1024].
  - Denominator: pair sums of P^T tiles on VectorE (bf16), quad sums
    on GpSimd (otherwise idle), then all-ones-stationary matmuls
    broadcast the column sums into a [128, 512]-per-half PSUM
    accumulator.  Normalize with reciprocal_approx_fast + multiply,
    DMA bf16 out^T [D, L] to DRAM.
  - Tail: the final Lk tile's exp is split into halves on ScalarE and
    p14/pair6 denominator matmuls are emitted at t=15 so the last
    d += ones*p15 -> recip -> mul -> DMA chain is as short as possible.
  - The host casts x/W to bf16 on the way in and transposes/upcasts
    out^T -> [L, D] f32 on the way out (pure layout/precision prep,
    like the batch scatter/gather).
Softmax max-subtraction is skipped: scores/sqrt(D) are ~N(0,1), so
exp() cannot overflow in fp32.
"""

import numpy as np

B, C, L, D = 8, 256, 2048, 128
F = 512          # Lq chunk
HALF = L // 2    # 1024
NP = 2           # passes (pairs of Lq chunks)
W2 = 2 * F       # 1024: width of paired tiles
LT = L // 128    # 16 Lk tiles
CK = C // 128    # 2 contraction chunks
SCALE = 1.0 / float(np.sqrt(D))

# Schraudolph bf16 exp: bits = round(s*ALPHA + BETA), bitcast int16->bf16
# approximates exp(s*SCALE).  -0.04305 centers the sawtooth error (+-3.3%).
ALPHA = 128.0 * SCALE * float(np.log2(np.e))
BETA = 128.0 * (127.0 - 0.04305)

# Which Lk tiles' exp runs on VectorE (per pass).  EVEN tiles only: the
# emission order per tile is scores -> exp -> av -> pair-add, so at even
# t the VectorE exp leads that tile's tree work in the queue and releases
# the score-PSUM slot promptly (at odd t it would queue behind the
# previous pair/quad adds, stalling the PE ~1us).  Tail tiles stay on
# ScalarE (t=0/1 gate the pipeline head; t=14/15 gate the tail).
# pass-1 starts at t=4 (not t=2): its first VectorE exp would otherwise
# queue behind pass-0's normalize chain (recip/mul) still draining on
# VectorE at the pass boundary
# Offload disabled: ScalarE's 32 exps (~33us) now fit under the PE wall
# (~42us) since the per-exp cost dropped; all-ACT removes both the
# VectorE-exp slot-release stalls and the approximation error entirely.
DVE_EXP = {0: (), 1: ()}

_COMPILED = None


def _build():
    import concourse.bass as bass
    import concourse.mybir as mybir
    import concourse.tile as tile
    from concourse import bacc
    from concourse.masks import make_identity
    from contextlib import ExitStack

    F32 = mybir.dt.float32
    BF16 = mybir.dt.bfloat16
    I16 = mybir.dt.int16
    AFT = mybir.ActivationFunctionType
    ALU = mybir.AluOpType
    ts = bass.ts

    nc = bacc.Bacc("TRN2", target_bir_lowering=False, debug=False, num_devices=8)

    xi_ext = nc.declare_dram_parameter("x_inner", [C, L], BF16, isOutput=False)
    xo_ext = nc.declare_dram_parameter("x_outer", [C, L], BF16, isOutput=False)
    w_ext = nc.declare_dram_parameter("W_all", [128, 3 * CK * D], BF16, isOutput=False)
    b_ext = nc.declare_dram_parameter("b_all", [D, 3], F32, isOutput=False)
    out_ext = nc.declare_dram_parameter("out", [D, L], BF16, isOutput=True)

    with tile.TileContext(nc) as tc:
        with ExitStack() as ctx:
            const = ctx.enter_context(tc.tile_pool(name="const", bufs=1))
            xin = ctx.enter_context(tc.tile_pool(name="xin", bufs=1))
            qkv = ctx.enter_context(tc.tile_pool(name="qkv", bufs=1))
            pts = ctx.enter_context(tc.tile_pool(name="pts", bufs=16))
            work = ctx.enter_context(tc.tile_pool(name="work", bufs=3))
            ps_s = ctx.enter_context(tc.tile_pool(name="ps_s", bufs=2, space="PSUM"))
            ps_av = ctx.enter_context(tc.tile_pool(name="ps_av", bufs=2, space="PSUM"))
            ps_d = ctx.enter_context(tc.tile_pool(name="ps_d", bufs=2, space="PSUM"))

            # ---- input DMAs first: h0 tiles on the sync queue (nothing
            # else competes), h1 tiles on gpsimd after the identity build.
            # Each tile is [128, (c=2), 1024]: partition p holds rows p and
            # 128+p of the [256, L] input -> 2KB contiguous DRAM runs.
            xo_t = [xin.tile([128, CK, HALF], BF16, tag=f"xo{h}", name=f"xo{h}")
                    for h in range(2)]
            xi_t = [xin.tile([128, CK, HALF], BF16, tag=f"xi{h}", name=f"xi{h}")
                    for h in range(2)]
            # weights/bias first on scalar queue (tiny; host pre-arranged
            # W as [128, 3*2*128] so each partition is one 1.5KB run)
            w_all = const.tile([128, 3, CK, D], BF16, tag="w")
            nc.scalar.dma_start(
                out=w_all[:],
                in_=w_ext[:].rearrange("p (w j d) -> p w j d", w=3, j=CK),
            )
            b_all = const.tile([D, 3], F32, tag="b")
            nc.scalar.dma_start(out=b_all[:], in_=b_ext[:])

            # All x DMAs on the sync queue IN PRIORITY ORDER: per-HW-queue
            # FIFO serializes them, so xo-h0 gets the full HBM bandwidth
            # and lands first (concurrent issue would round-robin all four
            # and finish them together ~4x later).  The h0 tiles are split
            # by c-chunk so the first projection matmul can start as soon
            # as chunk 0 lands.
            for c in range(CK):
                nc.sync.dma_start(
                    out=xo_t[0][:, c, :],
                    in_=xo_ext[c * 128:(c + 1) * 128, 0:HALF],
                )
            for c in range(CK):
                nc.sync.dma_start(
                    out=xi_t[0][:, c, :],
                    in_=xi_ext[c * 128:(c + 1) * 128, 0:HALF],
                )
            nc.sync.dma_start(
                out=xo_t[1][:],
                in_=xo_ext[:, HALF:].rearrange("(j p) l -> p j l", p=128),
            )
            nc.sync.dma_start(
                out=xi_t[1][:],
                in_=xi_ext[:, HALF:].rearrange("(j p) l -> p j l", p=128),
            )

            # vector: constants needed by the PE warm-up
            ones = const.tile([128, 128], BF16, tag="ones")
            nc.vector.memset(ones[:], 1.0)
            warm_src = const.tile([128, F], BF16, tag="warm")
            nc.vector.memset(warm_src[:], 0.0)

            # gpsimd: identity for PE transposes
            ident = const.tile([128, 128], BF16, tag="ident")
            make_identity(nc, ident[:])

            # ---- PE warm-up: a few dummy matmuls while the first input
            # DMA is in flight so HAM un-throttles before real work.
            wp = ps_d.tile([128, F], F32, tag="d", name="warm_ps")
            for _ in range(5):
                nc.tensor.matmul(wp[:], ones[:], warm_src[:], start=True, stop=True)

            # ---- projections ----------------------------------------------
            def xs_ap(tiles, c, pr):
                return tiles[pr][:, c, :]

            def project_pair(w, b, xs, pr, tag, pool=None, bias_eng="dve",
                             bias_halves=False):
                pool = pool or ps_s
                sb = qkv.tile([128, W2], BF16, tag=f"{tag}{pr}", name=f"{tag}{pr}")

                def bias_copy(dst, src):
                    if bias_eng == "act":
                        nc.scalar.activation(
                            dst, src, AFT.Identity, bias=b_all[:, b:b + 1]
                        )
                    else:
                        nc.vector.tensor_scalar_add(dst, src, b_all[:, b:b + 1])

                if pool is ps_s:
                    ps = pool.tile([128, W2], F32, tag="s", name="proj_ps")
                    for c in range(CK):
                        for h in range(2):
                            nc.tensor.matmul(
                                ps[:, ts(h, F)],
                                w_all[:, w, c, :],
                                xs_ap(xs, c, pr)[:, ts(h, F)],
                                start=(c == 0), stop=(c == CK - 1),
                            )
                    if bias_halves:
                        # per-half bias so downstream consumers of the h0
                        # half start one PSUM-half earlier (head critical
                        # chain: Q bias -> scores t0 -> exp0)
                        for h in range(2):
                            bias_copy(sb[:, ts(h, F)], ps[:, ts(h, F)])
                    else:
                        bias_copy(sb[:], ps[:])
                else:
                    # two 1-bank halves (fits the ps_d slot size)
                    for h in range(2):
                        ps = pool.tile([128, F], F32, tag="d", name="proj_ph")
                        for c in range(CK):
                            nc.tensor.matmul(
                                ps[:],
                                w_all[:, w, c, :],
                                xs_ap(xs, c, pr)[:, ts(h, F)],
                                start=(c == 0), stop=(c == CK - 1),
                            )
                        bias_copy(sb[:, ts(h, F)], ps[:])
                return sb

            ktP, vtP, qtP = [None, None], [None, None], [None, None]
            v_sb = [None] * LT

            def make_v_tiles(g, pool, ptag):
                tp_all = pool.tile([128, 8 * 128], BF16, tag=ptag, name="tp_all")
                for j in range(8):
                    t = g * 8 + j
                    nc.tensor.transpose(
                        tp_all[:, ts(j, 128)],
                        vtP[t // 8][:, (t % 8) * 128:(t % 8 + 1) * 128],
                        ident[:],
                    )
                for j in range(8):
                    t = g * 8 + j
                    vv = qkv.tile([128, 128], BF16, tag=f"v{t}", name=f"v{t}")
                    nc.vector.tensor_copy(vv[:], tp_all[:, ts(j, 128)])
                    v_sb[t] = vv

            def kslice(t):
                return ktP[t // 8][:, (t % 8) * 128:(t % 8 + 1) * 128]

            # Pair-0 projections: K first (earliest DMA), then Q (gated on
            # xi h0), then V + transposes.  K/Q bias on ScalarE (idle in
            # the head), V bias on VectorE.
            # K bias on VectorE (idle in the head) so ScalarE only carries
            # the Q bias before the first exp; the V projection/transposes
            # are emitted after the first score tile (AV first needs V at
            # t=1), shortening the critical chain to exp0.
            ktP[0] = project_pair(1, 1, xo_t, 0, "kt_", bias_eng="dve",
                                  bias_halves=True)
            qtP[0] = project_pair(0, 0, xi_t, 0, "qt_", bias_eng="act",
                                  bias_halves=True)

            # ---- attention: 2 passes over Lq chunk pairs -------------------
            state = {}

            def init_pass(pr):
                state[pr] = dict(
                    av=[ps_av.tile([128, F], F32, tag="av", name=f"av{h}")
                        for h in range(2)],
                    d=None, p_tiles=[], pair_sums=[], quad_sums=[],
                )

            def do_av(pr, t, only_h=None):
                st = state[pr]
                for h in range(2) if only_h is None else (only_h,):
                    nc.tensor.matmul(
                        st["av"][h][:], v_sb[t][:],
                        st["p_tiles"][t][:, ts(h, F)],
                        start=(t == 0), stop=(t == LT - 1),
                    )

            def do_pair_add(pr, m):
                st = state[pr]
                # pair sums on VectorE (they recycle P-tile pool slots, so
                # they must keep up with the exp rate); quad sums on GpSimd
                # (otherwise idle, and the PE d-matmuls that consume them
                # are scheduled 2+ tiles later).  The pass-0 boundary quad
                # stays on VectorE so the pass-0 normalize isn't delayed.
                sm = pts.tile([128, W2], BF16, tag="p", name="sm")
                nc.vector.tensor_add(
                    sm[:], st["p_tiles"][2 * m][:], st["p_tiles"][2 * m + 1][:]
                )
                st["pair_sums"].append(sm)
                if m % 2 == 1 and not (pr == 1 and m >= LT // 2 - 2):
                    # quads on VectorE: 0.84us right behind the pair add,
                    # so the PE d-matmuls 2 tiles later never stall (GpSimd
                    # at 2.1us/add left them ~1us late each)
                    q = pts.tile([128, W2], BF16, tag="p", name="quad")
                    nc.vector.tensor_add(
                        q[:], st["pair_sums"][m - 1][:], st["pair_sums"][m][:]
                    )
                    st["quad_sums"].append(q)

            def do_dn(pr, m):
                st = state[pr]
                if st["d"] is None:
                    st["d"] = [ps_d.tile([128, F], F32, tag="d", name=f"d{h}")
                               for h in range(2)]
                for h in range(2):
                    nc.tensor.matmul(
                        st["d"][h][:], ones[:], st["quad_sums"][m][:, ts(h, F)],
                        start=(m == 0), stop=(pr == 0 and m == LT // 4 - 1),
                    )

            def do_exp(pr, t, p_sb, s_ps):
                if t in DVE_EXP[pr]:
                    nc.vector.tensor_scalar(
                        out=p_sb[:].bitcast(I16), in0=s_ps[:],
                        scalar1=ALPHA, scalar2=BETA,
                        op0=ALU.mult, op1=ALU.add,
                    )
                elif (pr == 1 and t == LT - 1) or (pr == 0 and t == 0):
                    # split the boundary exps: at the tail the h0
                    # denominator chain starts one half-exp earlier; at the
                    # head exp0-h0 starts as soon as the h0 scores (gated
                    # on the per-half Q bias) are ready
                    for h in range(2):
                        nc.scalar.activation(
                            p_sb[:, ts(h, F)], s_ps[:, ts(h, F)],
                            AFT.Exp, scale=SCALE,
                        )
                else:
                    nc.scalar.activation(p_sb[:], s_ps[:], AFT.Exp, scale=SCALE)

            def emit_att(pr, t_lo, t_hi):
                st = state[pr]
                for t in range(t_lo, t_hi):
                    s_ps = ps_s.tile([128, W2], F32, tag="s", name="s_ps")
                    for h in range(2):
                        nc.tensor.matmul(
                            s_ps[:, ts(h, F)], kslice(t), qtP[pr][:, ts(h, F)],
                            start=True, stop=True,
                        )
                    p_sb = pts.tile([128, W2], BF16, tag="p", name="p_sb")
                    do_exp(pr, t, p_sb, s_ps)
                    st["p_tiles"].append(p_sb)
                    if t >= 1:
                        do_av(pr, t - 1)
                    if t >= 2 and t % 2 == 0:
                        do_pair_add(pr, t // 2 - 1)
                    if pr == 1:
                        # quad m is formed at t = 4m + 4 on VectorE; consume
                        # it ONE tile later so the PE d-matmul never queues
                        # behind the same-iteration pair->quad chain
                        if t >= 5 and (t - 5) % 4 == 0 and (t - 5) // 4 < 3:
                            do_dn(pr, (t - 5) // 4)
                        if t == LT - 1:
                            # pair6 (tiles 12,13) and p14 feed the
                            # denominator now so only p15 remains at the end
                            for h in range(2):
                                nc.tensor.matmul(
                                    st["d"][h][:], ones[:],
                                    st["pair_sums"][LT // 2 - 2][:, ts(h, F)],
                                    start=False, stop=False,
                                )
                            for h in range(2):
                                nc.tensor.matmul(
                                    st["d"][h][:], ones[:],
                                    st["p_tiles"][LT - 2][:, ts(h, F)],
                                    start=False, stop=False,
                                )

            def normalize_half(pr, h, st, chunks=1):
                recip = work.tile([128, F], F32, tag="recip", name="recip")
                avn = work.tile([128, F], BF16, tag="avn", name="avn")
                nc.vector.reciprocal_approx_fast(recip[:], st["d"][h][:])
                cw = F // chunks
                for i in range(chunks):
                    nc.vector.tensor_mul(
                        avn[:, ts(i, cw)],
                        st["av"][h][:, ts(i, cw)], recip[:, ts(i, cw)],
                    )
                    base = (2 * pr + h) * F + i * cw
                    nc.sync.dma_start(
                        out=out_ext[:, base:base + cw], in_=avn[:, ts(i, cw)]
                    )

            def finish_pass(pr):
                st = state[pr]
                do_av(pr, LT - 1)
                if pr == 0:
                    do_pair_add(pr, LT // 2 - 1)
                    # all of pass-0's denominator matmuls land here: they
                    # fill the pass-boundary PE gap and overlap pass-1.
                    for m in range(LT // 4):
                        do_dn(pr, m)
                    for h in range(2):
                        normalize_half(pr, h, st)
                else:
                    # tail: only p15 remains; per-half add -> d -> recip ->
                    # mul -> DMA, h0 first, with the mul/DMA split in two
                    # chunks so the output transfer overlaps the last mul
                    for h in range(2):
                        nc.tensor.matmul(
                            st["d"][h][:], ones[:],
                            st["p_tiles"][LT - 1][:, ts(h, F)],
                            start=False, stop=True,
                        )
                        normalize_half(pr, h, st, chunks=2)

            # mid-kernel projection biases ride on ScalarE (62%-busy vs
            # VectorE's denominator-tree load) as Identity-with-bias
            init_pass(0)
            emit_att(0, 0, 1)
            vtP[0] = project_pair(2, 2, xo_t, 0, "vt_", bias_eng="dve")
            make_v_tiles(0, ps_d, "d")
            emit_att(0, 1, 8)
            ktP[1] = project_pair(1, 1, xo_t, 1, "kt_", pool=ps_d, bias_eng="act")
            vtP[1] = project_pair(2, 2, xo_t, 1, "vt_", pool=ps_d, bias_eng="act")
            make_v_tiles(1, ps_d, "d")
            emit_att(0, 8, LT)
            qtP[1] = project_pair(0, 0, xi_t, 1, "qt_", bias_eng="act")
            finish_pass(0)
            init_pass(1)
            emit_att(1, 0, LT)
            finish_pass(1)

    nc.compile()
    return nc


def _in_maps(inputs):
    import ml_dtypes

    bf16 = ml_dtypes.bfloat16
    x_inner = np.ascontiguousarray(np.asarray(inputs["x_inner"]).astype(bf16))
    x_outer = np.ascontiguousarray(np.asarray(inputs["x_outer"]).astype(bf16))
    # [w, c, d] -> [p, w, j, d] with c = j*128 + p, flattened to [128, 768]
    # so the on-device tile DMA reads one contiguous 1.5KB run per partition
    w_wcd = np.stack([
        np.asarray(inputs["Wq"]).astype(np.float32).T,
        np.asarray(inputs["Wk"]).astype(np.float32).T,
        np.asarray(inputs["Wv"]).astype(np.float32).T,
    ])
    w_all = np.ascontiguousarray(
        w_wcd.reshape(3, CK, 128, D).transpose(2, 0, 1, 3).reshape(128, 3 * CK * D)
    ).astype(bf16)
    b_all = np.ascontiguousarray(np.stack([
        np.asarray(inputs["bq"], dtype=np.float32),
        np.asarray(inputs["bk"], dtype=np.float32),
        np.asarray(inputs["bv"], dtype=np.float32),
    ], axis=1))
    return [
        {
            "x_inner": x_inner[b],
            "x_outer": x_outer[b],
            "W_all": w_all,
            "b_all": b_all,
        }
        for b in range(B)
    ]


def kernel(**inputs):
    global _COMPILED
    from concourse.bass_utils import run_bass_kernel_spmd

    if _COMPILED is None:
        _COMPILED = _build()
    in_maps = _in_maps(inputs)
    res = run_bass_kernel_spmd(_COMPILED, in_maps, core_ids=list(range(B)))
    # device emits bf16 out^T [D, L]; transpose/upcast on host (pure layout)
    return np.stack(
        [res.results[b]["out"].T.astype(np.float32) for b in range(B)]
    )
